# revision 42
# baseline (speedup 1.0000x reference)
"""Multi-head cross-attention Trainium2 kernel (8-core SPMD), v12.

Sharding: 2 batch groups x 4 cores. Core c handles batch b = c // 4 and
heads [4*(c%4), 4*(c%4)+4) (= 2 head-pairs "mh"). Each core computes its
4 heads' attention and a partial output projection (row-sharded Wp); the
host sums 4 partials per batch (fp16 partials, fp32 accumulation).

Design (219.4 us vs 260.9 us fp16 baseline; ACT-exp window is the
roofline at ~143 us; phases: warmup ~51 DMA+proj, window ~146, tail ~29):
  - Q/K projections in fp8e4 DoubleRow: contraction 256 (two 128-deep
    k-tiles per instr) at the same stream rate -> half the projection
    instructions. Wq,Wk,bq,bk pre-scaled x32 so q,k use the fp8e4
    normal range; the exp scale absorbs the 1024x. (Measured on HW:
    DR doubles contraction per instr, NOT stream rate; 64-partition DR
    is slower than fp16, so QK^T itself stays fp16.)
  - V path fp16 end-to-end (v-quant error does not average out in the
    softmax mean). V is produced directly in [s, d] layout by using the
    e^T tile as lhsT - no PE transposes; k-major accumulation over 16
    packed PSUM accumulators (memset + acc-mode matmuls) so each
    arriving eT16 chunk feeds 16 matmuls with no chunk-wait bubbles;
    bias added at evac via a partition_broadcast'd bias row.
  - Attention per (head, T-half): 2x fp16 QK [128,512] -> one exp ACT
    instr on [128,1024] PSUM -> 2x fp16 AV accumulating [128,1024]
    (ones column at VA col 0 puts the softmax denominator on PSUM
    partition 0, U on partitions 64-127).
  - Normalize off the PE: DVE reciprocal of the denom row (partition 0
    only - custom DVE ops misbehave at base partition 64 on HW),
    gpsimd partition_broadcast, DVE multiply; odd heads write UN
    directly, even heads partition-shift via DMA.
  - All projections run eagerly in the DMA-bound warmup; the ACT window
    carries only QK/AV/outproj + normalize via deferred queues.
    PSUM: att-tag 3x2 banks + uh-tag 1x2 banks.
"""

import os
import numpy as np
import ml_dtypes
from contextlib import ExitStack
from collections import deque

import concourse.bass as bass
import concourse.bacc as bacc
import concourse.tile as tile
from concourse import mybir
from concourse.bass_utils import run_bass_kernel_spmd

F32 = mybir.dt.float32
F16 = mybir.dt.float16
F8 = mybir.dt.float8e4
AF = mybir.ActivationFunctionType
DR = mybir.MatmulPerfMode.DoubleRow

B, T, S, C = 2, 2048, 2048, 1024
H, HD = 16, 64
NCORES = 8
HPC = 4            # heads per core
MHN = 2            # head-pairs per core
KC = C // 128      # 8 contraction tiles
STILES = S // 128  # 16
WSCALE = 32.0      # host pre-scale on Wq/Wk/bq/bk
EXP_SCALE = 1.0 / (np.sqrt(C) * WSCALE * WSCALE)  # 2^-15

LAST_RESULTS = None
_NC_CACHE = None


def _build_nc():
    nc = bacc.Bacc()

    xT8 = nc.declare_dram_parameter("xT8", [128, KC, T], F8, isOutput=False)
    eT8 = nc.declare_dram_parameter("eT8", [128, KC, S], F8, isOutput=False)
    eT16 = nc.declare_dram_parameter("eT16", [128, KC, S], F16, isOutput=False)
    Wq8 = nc.declare_dram_parameter("Wq8", [128, KC, 256], F8, isOutput=False)
    Wk8 = nc.declare_dram_parameter("Wk8", [128, KC, 256], F8, isOutput=False)
    Wv16 = nc.declare_dram_parameter("Wv16", [128, KC, 256], F16, isOutput=False)
    b6 = nc.declare_dram_parameter("b6", [128, 6], F32, isOutput=False)
    bvr = nc.declare_dram_parameter("bvr", [1, 256], F32, isOutput=False)
    WpT4 = nc.declare_dram_parameter("WpT4", [128, 2, C], F16, isOutput=False)
    y = nc.declare_dram_parameter("y", [T, C], F16, isOutput=True)

    with tile.TileContext(nc) as tc, ExitStack() as ctx:
        consts = ctx.enter_context(tc.tile_pool(name="consts", bufs=1))
        wpool = ctx.enter_context(tc.tile_pool(name="wts", bufs=1))
        qkvp = ctx.enter_context(tc.tile_pool(name="qkvt", bufs=2))
        vtsp = ctx.enter_context(tc.tile_pool(name="vts", bufs=2))
        vap = ctx.enter_context(tc.tile_pool(name="vaug", bufs=2))
        epool = ctx.enter_context(tc.tile_pool(name="esb", bufs=4))
        unp = ctx.enter_context(tc.tile_pool(name="unorm", bufs=2))
        usbp = ctx.enter_context(tc.tile_pool(name="usb", bufs=3))
        dnp = ctx.enter_context(tc.tile_pool(name="denom", bufs=2))
        psp = ctx.enter_context(tc.tile_pool(name="ps", bufs=2, space="PSUM"))

        # ---- constants / weights / inputs ----


        # Tiny bias DMAs first (the sync DMA queue is in-order; these
        # gate every projection evac), then bulk ordered by first
        # consumer: Wv+eT16 (V pass, the warmup long pole), eT8, xT8.
        b6sb = consts.tile([128, 6], F32, tag="b6", name="b6sb")
        nc.sync.dma_start(out=b6sb, in_=b6[:, :])
        # bsb[x][:, mh] view: q cols 0-1, k cols 2-3
        bsb = {"q": b6sb[:, 0:2], "k": b6sb[:, 2:4]}
        bvr_sb = consts.tile([1, 256], F32, tag="bvr", name="bvrsb")
        nc.sync.dma_start(out=bvr_sb, in_=bvr[:, :])
        bvb = consts.tile([128, 256], F32, tag="bvb", name="bvb")
        nc.gpsimd.partition_broadcast(bvb, bvr_sb, channels=128)
        w16v = wpool.tile([128, KC, 256], F16, tag="wv", name="wv16sb")
        nc.sync.dma_start(out=w16v, in_=Wv16[:, :, :])
        et16 = wpool.tile([128, KC, S], F16, tag="e16")
        nc.sync.dma_start(out=et16[:, 0, 0:1024], in_=eT16[:, 0, 0:1024])
        nc.sync.dma_start(out=et16[:, 0, 1024:2048], in_=eT16[:, 0, 1024:2048])
        for k in range(1, KC):
            nc.sync.dma_start(out=et16[:, k, :], in_=eT16[:, k, :])
        w8k = wpool.tile([128, KC, 256], F8, tag="wk", name="wk8sb")
        nc.sync.dma_start(out=w8k, in_=Wk8[:, :, :])
        et8 = wpool.tile([128, KC, S], F8, tag="e8")
        for k in range(KC):
            nc.sync.dma_start(out=et8[:, k, :], in_=eT8[:, k, :])
        w8q = wpool.tile([128, KC, 256], F8, tag="wq", name="wq8sb")
        nc.sync.dma_start(out=w8q, in_=Wq8[:, :, :])
        xt8 = wpool.tile([128, KC, T], F8, tag="xt")
        for k in range(KC):
            nc.sync.dma_start(out=xt8[:, k, :], in_=xT8[:, k, :])
        wpt = wpool.tile([128, 2, C], F16, tag="wpt")
        nc.sync.dma_start(out=wpt, in_=WpT4[:, :, :])

        # Q^T/K^T in fp16 (64-partition fp8-DR matmuls measure SLOWER
        # than fp16 on HW; fp8 only pays off at 128-part full slabs).
        QT = [qkvp.tile([128, T], F16, tag="qt", name=f"QT_{i}")
              for i in range(MHN)]
        KT = [qkvp.tile([128, S], F16, tag="kt", name=f"KT_{i}")
              for i in range(MHN)]

        # VA columns per head: [ones@0 | zeros@1-63 | v@64-127] so the
        # softmax denominator lands on PSUM partition 0 (the only source
        # partition_broadcast supports) and U lands on partitions 64-127
        # (a legal engine base partition).
        VA = [vap.tile([128, STILES, 256], F16, tag="va", name=f"va{i}")
              for i in range(MHN)]
        for i in range(MHN):
            for hh in range(2):
                nc.gpsimd.memset(VA[i][:, :, hh * 128:hh * 128 + 1], 1.0)
                nc.gpsimd.memset(VA[i][:, :, hh * 128 + 1:hh * 128 + 64], 0.0)

        def q_pass(mh, halves=(0, 1)):
            """Generator of closures: Q^T (fp8 DoubleRow) for one pair."""
            for half in halves:
                hsl = slice(half * 1024, (half + 1) * 1024)
                state = {}

                def mk_alloc(state=state, mh=mh, half=half):
                    state["ps"] = psp.tile([128, 1024], F32, tag="att",
                                           bufs=3, name=f"qps{mh}_{half}")

                yield mk_alloc
                for j in range(4):
                    def mk_j(j=j, state=state, mh=mh, half=half):
                        for n in range(2):
                            csl = slice(half * 1024 + n * 512,
                                        half * 1024 + n * 512 + 512)
                            nc.tensor.matmul(
                                state["ps"][:, n * 512:(n + 1) * 512],
                                w8q[:, 2 * j:2 * j + 2,
                                    mh * 128:(mh + 1) * 128],
                                xt8[:, 2 * j:2 * j + 2, csl],
                                start=(j == 0), stop=(j == 3), perf_mode=DR)

                    yield mk_j

                def mk_evac(state=state, mh=mh, hsl=hsl):
                    nc.vector.tensor_scalar_add(
                        out=QT[mh][:, hsl], in0=state["ps"],
                        scalar1=bsb["q"][:, mh:mh + 1])

                yield mk_evac

        def k_pass(mh):
            """Generator of closures: K^T (fp8 DoubleRow) for one pair."""
            for half in range(2):
                hsl = slice(half * 1024, (half + 1) * 1024)
                state = {}

                def mk_alloc(state=state, mh=mh, half=half):
                    state["ps"] = psp.tile([128, 1024], F32, tag="att",
                                           bufs=3, name=f"kps{mh}_{half}")

                yield mk_alloc
                for j in range(4):
                    def mk_j(j=j, state=state, mh=mh, half=half):
                        for n in range(2):
                            csl = slice(half * 1024 + n * 512,
                                        half * 1024 + n * 512 + 512)
                            nc.tensor.matmul(
                                state["ps"][:, n * 512:(n + 1) * 512],
                                w8k[:, 2 * j:2 * j + 2,
                                    mh * 128:(mh + 1) * 128],
                                et8[:, 2 * j:2 * j + 2, csl],
                                start=(j == 0), stop=(j == 3), perf_mode=DR)

                    yield mk_j

                def mk_evac(state=state, mh=mh, hsl=hsl):
                    nc.vector.tensor_scalar_add(
                        out=KT[mh][:, hsl], in0=state["ps"],
                        scalar1=bsb["k"][:, mh:mh + 1])

                yield mk_evac

        def v_pass():
            """Generator of closures: V in [s, d] layout for all 4 heads.
            k-major: each arriving eT16 chunk feeds 16 matmuls (one per
            s-tile accumulator), so the PE never waits on a chunk. The 16
            [128,256] accumulators pack 4-per-slot into the 4 PSUM slots.
            Per-slot evacs run as soon as that slot's k=7 matmuls retire."""
            state = {}

            def mk_alloc(state=state):
                ps = []
                for i in range(4):
                    tg, bf = ("att", 3) if i < 3 else ("uh", 1)
                    t = psp.tile([128, 1024], F32, tag=tg, bufs=bf,
                                 name=f"vps{i}")
                    nc.vector.memset(t, 0.0)
                    ps.append(t)
                state["ps"] = ps

            yield mk_alloc

            def mk_k(k, state=state):
                # accumulate-mode matmuls onto the memset tiles: four
                # 256-col accumulators share each tile, which the PSUM
                # group tracker cannot express (one group per region).
                for s in range(STILES):
                    q = s % 4
                    nc.tensor.matmul(
                        state["ps"][s // 4][:, q * 256:(q + 1) * 256],
                        et16[:, k, s * 128:(s + 1) * 128],
                        w16v[:, k, :], start=False, stop=(k == KC - 1),
                        skip_group_check=True)

            for k in range(KC):
                yield (lambda k=k, f=mk_k: f(k))

            def mk_evac(i, state=state):
                for s in range(i * 4, i * 4 + 4):
                    q = s % 4
                    for mh in range(MHN):
                        for hh in range(2):
                            c = q * 256 + (mh * 2 + hh) * 64
                            nc.vector.tensor_add(
                                VA[mh][:, s, hh * 128 + 64:hh * 128 + 128],
                                state["ps"][i][:, c:c + 64],
                                bvb[:, (mh * 2 + hh) * 64:
                                    (mh * 2 + hh) * 64 + 64])

            for i in range(4):
                yield (lambda i=i, f=mk_evac: f(i))

        UN = [unp.tile([128, T], F16, tag="un", name=f"UN{i}")
              for i in range(MHN)]

        y_r = y.rearrange("(tt j p) o -> tt p j o", p=128, j=4)

        def outproj_work(tqs):
            """Deferred: partial out-projection for given t-quarters."""
            for tq in tqs:
                state = {}

                def mk_alloc(state=state, tq=tq):
                    state["ysb"] = dnp.tile([128, 4, 1024], F16, tag="ysb",
                                            bufs=2, name=f"ysb{tq}")

                yield mk_alloc

                def mk_tile(j, state=state, tq=tq):
                    t = tq * 4 + j
                    y_ps = psp.tile([128, 1024], F32, tag="att",
                                    bufs=3, name=f"yps{t}")
                    for mh in range(MHN):
                        for n in range(2):
                            nc.tensor.matmul(
                                y_ps[:, n * 512:(n + 1) * 512],
                                UN[mh][:, t * 128:(t + 1) * 128],
                                wpt[:, mh, n * 512:(n + 1) * 512],
                                start=(mh == 0), stop=(mh == MHN - 1))
                    nc.vector.tensor_copy(state["ysb"][:, j, :], y_ps)

                def mk_dma(hf, state=state, tq=tq):
                    sl = slice(hf * 2, hf * 2 + 2)
                    nc.sync.dma_start(out=y_r[tq][:, sl],
                                      in_=state["ysb"][:, sl])

                yield (lambda f=mk_tile: f(0))
                yield (lambda f=mk_tile: f(1))
                yield (lambda f=mk_dma: f(0))
                yield (lambda f=mk_tile: f(2))
                yield (lambda f=mk_tile: f(3))
                yield (lambda f=mk_dma: f(1))

        def normalize_work(mh, hh, half, uh, c0=0, cw=1024):
            """Deferred: evac U, gpsimd-broadcast denom (from p0), recip,
            divide on partitions 1-64, DMA partition-shift into UN."""
            hsl = slice(half * 1024 + c0, half * 1024 + c0 + cw)
            csl = slice(0, cw)
            state = {}

            def mk_evac():
                usb = usbp.tile([128, 1024], F32, tag="usb",
                                name=f"usb{mh}_{hh}_{half}_{c0}")
                nc.vector.tensor_copy(usb[:, csl], uh[:, c0:c0 + cw])
                state["usb"] = usb

            def mk_recip():
                # custom DVE ops misbehave at base partition 64 on HW:
                # reciprocal the single denom row at base 0, THEN broadcast.
                rrow = dnp.tile([1, 1024], F32, tag="rrow", bufs=2,
                                name=f"rrow{mh}_{hh}_{half}_{c0}")
                nc.vector.reciprocal_approx_fast(rrow[:, csl],
                                                 state["usb"][0:1, csl])
                state["rrow"] = rrow

            def mk_bcast():
                rbc = dnp.tile([128, 1024], F32, tag="rbc", bufs=2,
                               name=f"rbc{mh}_{hh}_{half}_{c0}")
                nc.gpsimd.partition_broadcast(rbc[:, csl],
                                              state["rrow"][:, csl],
                                              channels=128)
                state["rbc"] = rbc

            def mk_div():
                usb, rbc = state["usb"], state["rbc"]
                if hh == 1:
                    # odd head rows live at UN partitions 64-127: the DVE
                    # mul (lanes 64-127) can write them directly.
                    nc.vector.tensor_mul(UN[mh][64:128, hsl],
                                         usb[64:128, csl], rbc[64:128, csl])
                else:
                    tmp1 = dnp.tile([128, 1024], F16, tag="tmp1", bufs=2,
                                    name=f"tmp1_{mh}_{hh}_{half}_{c0}")
                    nc.vector.tensor_mul(tmp1[64:128, csl],
                                         usb[64:128, csl], rbc[64:128, csl])
                    nc.gpsimd.dma_start(
                        out=UN[mh][0:64, hsl], in_=tmp1[64:128, csl])

            yield mk_evac
            yield mk_recip
            yield mk_bcast
            yield mk_div

        def attention_all(seq, fast, bulk, outproj_after):
            """All attention blocks flattened into one 128-iteration
            pipeline: QK(i+1)/exp(i+1) issue before AV(i), ACROSS block
            boundaries, so the ACT stream never sees a boundary bubble."""
            prev_av = None
            prev_norm = None   # (mh, hh, half, uh) of the completed block
            uh = None
            for it in range(len(seq) * STILES):
                blk, s = divmod(it, STILES)
                mh, hh, half = seq[blk]
                psl = slice(hh * 64, hh * 64 + 64)
                if s == 0:
                    uh = psp.tile([128, 1024], F32, tag="uh",
                                  bufs=1, name=f"uh{blk}")
                att = psp.tile([128, 1024], F32, tag="att",
                               bufs=3, name=f"att_{blk}_{s}")
                for n in range(2):
                    csl = slice(half * 1024 + n * 512,
                                half * 1024 + n * 512 + 512)
                    nc.tensor.matmul(att[:, n * 512:(n + 1) * 512],
                                     KT[mh][psl, s * 128:(s + 1) * 128],
                                     QT[mh][psl, csl],
                                     start=True, stop=True)
                ej = epool.tile([128, 1024], F16, tag="e",
                                name=f"e_{blk}_{s}")
                nc.scalar.activation(ej, att, AF.Exp, scale=float(EXP_SCALE))
                if prev_av is not None:
                    prev_av()
                if prev_norm is not None:
                    # previous block finished at prev_av above: evac its
                    # uh inline (frees the single uh slot), queue the rest
                    ngen = normalize_work(*prev_norm)
                    next(ngen)()
                    fast.extend(ngen)
                    pblk = blk - 1
                    if seq[pblk] == outproj_after:
                        fast.extend(outproj_work((0, 1)))
                    prev_norm = None
                if fast:
                    fast.popleft()()
                elif bulk:
                    bulk.popleft()()

                def mk_av(s=s, ej=ej, uh=uh, mh=mh, hh=hh):
                    for n in range(2):
                        nc.tensor.matmul(
                            uh[:, n * 512:(n + 1) * 512],
                            VA[mh][:, s, hh * 128:hh * 128 + 128],
                            ej[:, n * 512:(n + 1) * 512],
                            start=(s == 0), stop=(s == STILES - 1))
                prev_av = mk_av
                if s == STILES - 1:
                    prev_norm = (mh, hh, half, uh)
            prev_av()
            ngen = normalize_work(*prev_norm)
            next(ngen)()
            fast.extend(ngen)
            return fast, bulk

        # All projections eager: the warmup is input-DMA-bound, so the
        # PE chews through both pairs' projections while data arrives,
        # leaving the ACT window free of deferred projection work.
        for gen in (v_pass(), k_pass(0), k_pass(1),
                    q_pass(0), q_pass(1)):
            for w in gen:
                w()
        bulk = deque()
        fast = deque()
        # (mh, hh, half): mh1 proj needed from block 5; half0's UN done
        # after block 6 -> outproj(0) hides in blocks 7-8.
        seq = [(0, 0, 0), (0, 1, 0), (1, 0, 0), (1, 1, 0),
               (0, 0, 1), (0, 1, 1), (1, 0, 1), (1, 1, 1)]
        fast, bulk = attention_all(seq, fast, bulk, outproj_after=(1, 1, 0))
        fast.extend(outproj_work((2, 3)))
        for q in (fast, bulk):
            while q:
                q.popleft()()

    nc.compile()
    return nc


def _get_nc():
    global _NC_CACHE
    if _NC_CACHE is None:
        _NC_CACHE = _build_nc()
    return _NC_CACHE


FP8 = ml_dtypes.float8_e4m3


def make_in_maps(e, x, Wq, bq, Wk, bk, Wv, bv, Wp):
    e = np.asarray(e, dtype=np.float32)
    x = np.asarray(x, dtype=np.float32)
    Wq, bq = np.asarray(Wq, np.float32), np.asarray(bq, np.float32)
    Wk, bk = np.asarray(Wk, np.float32), np.asarray(bk, np.float32)
    Wv, bv = np.asarray(Wv, np.float32), np.asarray(bv, np.float32)
    Wp = np.asarray(Wp, np.float32)

    def swiz(a2d, dt):  # [C, N] -> [128, KC, N] partition-major
        Cd, N = a2d.shape
        return np.ascontiguousarray(
            a2d.reshape(KC, 128, N).transpose(1, 0, 2).astype(dt))

    xT8s = [swiz(x[b].T, FP8) for b in range(B)]
    eT8s = [swiz(e[b].T, FP8) for b in range(B)]
    eT16s = [swiz(e[b].T, np.float16) for b in range(B)]
    in_maps = []
    for c in range(NCORES):
        b = c // 4
        h0 = (c % 4) * HPC
        cs = h0 * HD
        wq = swiz(WSCALE * Wq[h0:h0 + HPC].transpose(1, 0, 2)
                  .reshape(C, HPC * HD), FP8)
        wk = swiz(WSCALE * Wk[h0:h0 + HPC].transpose(1, 0, 2)
                  .reshape(C, HPC * HD), FP8)
        wv = swiz(Wv[h0:h0 + HPC].transpose(1, 0, 2)
                  .reshape(C, HPC * HD), np.float16)
        b6 = np.stack([WSCALE * bq[h0:h0 + HPC].reshape(2, 128),
                       WSCALE * bk[h0:h0 + HPC].reshape(2, 128),
                       bv[h0:h0 + HPC].reshape(2, 128)])  # [3, 2, 128]
        b6 = np.ascontiguousarray(
            b6.reshape(6, 128).T.astype(np.float32))      # [128, 6]
        wpt = np.ascontiguousarray(
            Wp[:, cs:cs + HPC * HD].T.astype(np.float16)
            .reshape(2, 128, C).transpose(1, 0, 2))       # [128, 2, C]
        in_maps.append({
            "xT8": xT8s[b], "eT8": eT8s[b], "eT16": eT16s[b],
            "Wq8": wq, "Wk8": wk, "Wv16": wv,
            "b6": b6, "WpT4": wpt,
            "bvr": np.ascontiguousarray(
                bv[h0:h0 + HPC].reshape(1, 256).astype(np.float32)),
        })
    return in_maps


def kernel(e, x, Wq, bq, Wk, bk, Wv, bv, Wp):
    global LAST_RESULTS
    nc = _get_nc()
    in_maps = make_in_maps(e, x, Wq, bq, Wk, bk, Wv, bv, Wp)
    res = run_bass_kernel_spmd(
        nc, in_maps, list(range(NCORES)),
        trace=bool(os.environ.get("BASS_TRACE")),
    )
    LAST_RESULTS = res
    out = np.zeros((B, T, C), dtype=np.float32)
    for c in range(NCORES):
        out[c // 4] += res.results[c]["y"].astype(np.float32)
    return out
